# revision 26
# baseline (speedup 1.0000x reference)
"""FeatureProcessingBlock Trainium kernel (bf16 pipeline, v5 — PE row-tiled
c-stage + bank-batched PSUM drains).

out = sum_t einsum('bcphqw,twW,thH,tcC->bCpHqW', x.reshape(B,C,P,64,Q,64), Ws, Hs, Cs)

Sharding: 8 cores = (B=4) x (H-halves=2); each core gets x[b, :, ph*256:(ph+1)*256, :]
a [C=48, 256, 512] slab = 4 p-blocks x 4 double-windows (dw = two adjacent
64x64 windows in a 128-wide w-slab).

Per-core pipeline (PSUM evacuation on ACT+DVE is the bottleneck; matmuls are
row-tiled so the PE stays off the critical path; drains are batched to large
free dims and split across both engines by measured rates):

  c-stage  (data-stationary, 2x PE row tiling): two concurrent quadrant
            matmuls T0/T8 contract c=48 for the even/odd h-parity:
            lhsT = x[(par) c48, wp128] chunk at partitions 64*par,
            rhs  = cstk [48, (t3 j24 cs2)=144] (plain Cs stack, no zeros)
            -> one 2-bank PSUM tile holds 3 hh x both parities, hh-interleaved
               so the drain is a single [128, 2par, 432] op
            -> drain/cast to Ybuf [128 (win,w), (t3, j24, cs2, h64)]  (cs-major)
  wT-stage (fused w-matmul + transpose; data-stationary): lhsT = Ybuf (t,j)-chunk
            [128 (win,w), 128 (cs,h)], rhs = blkdiag(Ws_t, Ws_t)
            -> PSUM [128 (cs,h), (win, W')] -- transposed for the h-stage
            -> drain/cast to ZT[t] bf16 (rows cs*64+h)
  h-stage  (t-sum in PSUM): lhsT = Hblk[t] (rows cs*64+h, cols (cs,H')),
            rhs = ZT[t] chunks, 3-matmul accumulation
            -> O [128 (cs, H'), (j8, W')] -> drain f32->bf16 -> DMA out
"""

import sys
import types

import numpy as np


def _ensure_ntff_hook_module():
    """concourse.bass_utils imports antenv.axon_hooks when BASS_TRACE is set;
    provide a fallback module (wired to the ctypes NTFF hook when available)
    so tracing degrades gracefully instead of crashing."""
    try:
        import antenv.axon_hooks  # noqa: F401

        return
    except ImportError:
        pass
    mod = types.ModuleType("antenv.axon_hooks")
    mod._hook = None
    mod.set_axon_ntff_profile_hook = lambda h: setattr(mod, "_hook", h)
    mod.get_axon_ntff_profile_hook = lambda: mod._hook
    sys.modules["antenv.axon_hooks"] = mod
    try:
        from trn_agent_boot.trn_boot import _ntff_profile_via_ctypes

        mod._hook = _ntff_profile_via_ctypes("/opt/axon/libaxon_pjrt.so")
    except Exception:
        pass


_ensure_ntff_hook_module()

B, C, H, W = 4, 48, 512, 512
T, WS = 3, 64
NCORES = 8
PH = H // 2    # 256 rows per core
NP = PH // 64  # 4 p-blocks

LAST_EXEC_NS = None
_CACHE = {}


def _build():
    import concourse.bacc as bacc
    import concourse.mybir as mybir
    from concourse.bass import MemorySpace
    from concourse.tile import TileContext

    F32 = mybir.dt.float32
    BF16 = mybir.dt.bfloat16

    nc = bacc.Bacc("TRN2", target_bir_lowering=False, debug=False, num_devices=NCORES)
    x = nc.dram_tensor("x", [C, PH, W], BF16, kind="ExternalInput")
    cstk = nc.dram_tensor("cstk", [128, T * 24 * 2], BF16, kind="ExternalInput")
    wblk = nc.dram_tensor("wblk", [T, 128, 64], BF16, kind="ExternalInput")
    hblk = nc.dram_tensor("hblk", [T, 128, 128], BF16, kind="ExternalInput")
    out = nc.dram_tensor("out", [C, PH, W], BF16, kind="ExternalOutput")

    with TileContext(nc) as tc:
        with (
            tc.tile_pool(name="consts", bufs=1) as consts,
            tc.tile_pool(name="xin", bufs=2) as xin,
            tc.tile_pool(name="xfirst", bufs=1) as xfirst,
            tc.tile_pool(name="ybuf", bufs=3) as ypool,
            tc.tile_pool(name="ztbuf", bufs=2) as ztpool,
            tc.tile_pool(name="obuf", bufs=2) as opool,
            tc.tile_pool(name="cps", bufs=2, space=MemorySpace.PSUM) as cps,
            tc.tile_pool(name="tps", bufs=2, space=MemorySpace.PSUM) as tps,
            tc.tile_pool(name="ops", bufs=1, space=MemorySpace.PSUM) as ops,
        ):
            # Cs stack, rows 0-47 (for quadrant T0) and 64-111 (T8)
            cstk_sb = consts.tile([128, T, 24, 2], BF16)
            nc.sync.dma_start(
                out=cstk_sb,
                in_=cstk[:, :].rearrange("k (t j s) -> k t j s", t=T, j=24),
            )
            wblk_sb = consts.tile([128, T, 64], BF16)
            hblk_sb = consts.tile([128, T, 128], BF16)
            wh_loaded = []

            def load_wh():
                nc.sync.dma_start(
                    out=wblk_sb, in_=wblk[:, :, :].rearrange("t k m -> k t m")
                )
                nc.sync.dma_start(
                    out=hblk_sb, in_=hblk[:, :, :].rearrange("t k m -> k t m")
                )
                wh_loaded.append(True)

            obs = {}

            # drain engine round-robin: drains are the global bottleneck, so
            # keep both engines fed; ACT is faster (1.2 vs 0.96 GHz) so it
            # gets a bigger share.
            def emit_c_stage(xt, yb, d):
                """c-stage for one block: 32 hh rows x 2 parities, grouped 3 hh
                per 2-bank PSUM tile (hh-interleaved cols) -> 11 batched drains."""
                wp0 = 128 * (d % 2)
                chunks = []
                groups = [(3 * g, min(3, 32 - 3 * g)) for g in range((32 + 2) // 3)]
                for gi, (hh0, nh) in enumerate(groups):
                    def c_group(hh0=hh0, nh=nh, gi=gi, yb=yb, wp0=wp0):
                        cp0 = cps.tile([128, 512], F32, tag="c0", name="cp0")
                        cp1 = cps.tile([128, 512], F32, tag="c1", name="cp1", bufs=1)
                        pvs = [cp0, cp1]
                        for i in range(nh):
                            xq, r = xt[hh0 + i]
                            for par in range(2):
                                v = pvs[par][:, 144 * i : 144 * i + 144].rearrange(
                                    "p (t j s) -> p t j s", t=T, j=24
                                )
                                nc.tensor.matmul(
                                    v,
                                    lhsT=xq[64 * par : 64 * par + 48, r, wp0 : wp0 + 128],
                                    rhs=cstk_sb[64 * par : 64 * par + 48],
                                    start=True,
                                    stop=True,
                                )
                        # batched per-parity drains (3D-fusable APs): src
                        # (tj, hh, cs) -> yb[..., h, cs] with h = 2*(hh0+hh)+par
                        for par in range(2):
                            src = pvs[par][:, 0 : 144 * nh].rearrange(
                                "p (h t j s) -> p (t j) h s", t=T, j=24, s=2
                            )
                            dst = yb[
                                :, :, :, 2 * hh0 + par : 2 * hh0 + 2 * nh : 2, :
                            ].rearrange("p t j h s -> p (t j) h s")
                            idx = 2 * gi + par
                            if idx % 5 < 3 and idx != 20:
                                nc.scalar.copy(out=dst, in_=src)
                            else:
                                nc.vector.tensor_copy(out=dst, in_=src)
                    chunks.append(c_group)
                return chunks

            def make_groups(p, d, yb, last=False):
                """wT + h + DMA emission callbacks for one block."""
                groups = []
                ztbs = [
                    ztpool.tile([128, 2, 24, 64], BF16, tag=f"zt{t}", name=f"ztb{t}")
                    for t in range(T)
                ]
                ob = obs[p]

                def wt_group(t, jq, yb=yb):
                    """row-tiled w-stage: win0 on quadrant rows 0-63 -> bank 0,
                    win1 on rows 64-127 -> bank 1 of a 2-bank tile; 8 j x 2 win
                    N=64 matmuls, then one FD512 drain per win."""
                    ztb = ztbs[t]
                    tp = tps.tile([128, 2, 8, 64], F32, tag="t")
                    for i in range(8):
                        for win in range(2):
                            nc.tensor.matmul(
                                tp[:, win, i],
                                lhsT=yb[64 * win : 64 * win + 64, t, 8 * jq + i],
                                rhs=wblk_sb[64 * win : 64 * win + 64, t, :],
                                start=True,
                                stop=True,
                            )
                    for win in range(2):
                        if (jq * 2 + win + t) % 4 == 1:
                            nc.scalar.copy(
                                out=ztb[:, win, 8 * jq : 8 * jq + 8, :],
                                in_=tp[:, win],
                            )
                        else:
                            nc.vector.tensor_copy(
                                out=ztb[:, win, 8 * jq : 8 * jq + 8, :],
                                in_=tp[:, win],
                            )

                def h_group(cc, win, d=d):
                    op = ops.tile([128, 8, 64], F32, tag="o")
                    for t in range(T):
                        nc.tensor.matmul(
                            op,
                            lhsT=hblk_sb[:, t, :],
                            rhs=ztbs[t][:, win, 8 * cc : 8 * cc + 8, :],
                            start=(t == 0),
                            stop=(t == T - 1),
                        )
                    nc.scalar.copy(out=ob[:, cc, :, d, win, :], in_=op)

                def out_dma(a, p=p, d=d):
                    for cs in range(2):
                        nc.sync.dma_start(
                            out=out[
                                16 * a + cs : 16 * a + 16 : 2,
                                64 * p : 64 * p + 64,
                                128 * d : 128 * d + 128,
                            ].rearrange("c h w -> h c w"),
                            in_=ob[
                                64 * cs : 64 * cs + 64, a, :, d
                            ].rearrange("p j win w -> p j (win w)"),
                        )

                # jq-major so h-stage cc=jq becomes ready right after its wT
                # groups; weave h/dma groups in so O-drains spread out
                import functools
                for jq in range(3):
                    for t in range(T):
                        groups.append(functools.partial(wt_group, t, jq))
                    groups.append(functools.partial(h_group, jq, 0))
                    groups.append(functools.partial(h_group, jq, 1))
                    groups.append(functools.partial(out_dma, jq))
                return groups

            prev_groups = []
            xts = {}
            blocks = [(p, d) for p in range(NP) for d in range(4)]

            def load_x(k):
                p, d = blocks[k]
                if d % 2 != 0:
                    return
                # ---- load half p-row in hh-pieces: [par@{0,64} c48, nh, 256 w]
                sizes = [3, 3, 2, 8, 8, 8] if k == 0 else [8, 8, 8, 8]
                hhmap = []
                hh0 = 0
                for q, nh in enumerate(sizes):
                    pool = xfirst if k == 0 else xin
                    xq = pool.tile(
                        [128, nh, 256], BF16, tag=f"x{q}", name=f"x{k}_{q}"
                    )
                    h0 = 64 * p + 2 * hh0
                    for par in range(2):
                        nc.sync.dma_start(
                            out=xq[64 * par : 64 * par + 48],
                            in_=x[
                                :,
                                h0 + par : h0 + 2 * nh : 2,
                                256 * (d // 2) : 256 * (d // 2) + 256,
                            ],
                        )
                    for r in range(nh):
                        hhmap.append((xq, r))
                    hh0 += nh
                xts[p, d // 2] = hhmap

            load_x(0)
            load_x(1)
            for k, (p, d) in enumerate(blocks):
                if k + 2 < len(blocks):
                    load_x(k + 2)
                if d % 2 == 0 and not wh_loaded:
                    load_wh()
                if d == 0:
                    obs[p] = opool.tile(
                        [128, 3, 8, 4, 2, 64], BF16, tag="ob", name=f"ob{p}"
                    )
                # Ybuf [128 (win,w), (t3, j24, h64, cs2)]
                yb = ypool.tile([128, T, 24, 64, 2], BF16, tag="y")
                chunks = emit_c_stage(xts[p, d // 2], yb, d)
                # proportionally interleave this block's c-stage with the
                # previous block's wT+h groups (no tail bursts)
                nc_, ng = len(chunks), len(prev_groups)
                for i in range(nc_):
                    chunks[i]()
                    for g in prev_groups[(i * ng) // nc_ : ((i + 1) * ng) // nc_]:
                        g()
                prev_groups = make_groups(p, d, yb)
            for g in prev_groups:
                g()

    nc.compile()
    return nc


def _get_nc():
    if "nc" not in _CACHE:
        _CACHE["nc"] = _build()
    return _CACHE["nc"]


def _prep_consts(Ws, Hs, Cs):
    import ml_dtypes

    bf = ml_dtypes.bfloat16
    # cstk [128, (t3, j24, cs2)]: rows 0-47 and 64-111 hold the plain Cs stack
    # (c, (t, j, cs)) with C' = 2j+cs
    cstk = np.zeros((128, T, 24, 2), np.float32)
    cst = Cs.transpose(1, 0, 2).reshape(C, T, 24, 2)
    cstk[0:48] = cst
    cstk[64:112] = cst
    cstk = cstk.reshape(128, T * 24 * 2)
    wblk = np.zeros((T, 128, 64), np.float32)
    hblk = np.zeros((T, 128, 128), np.float32)
    for t in range(T):
        wblk[t, 0:64] = Ws[t]
        wblk[t, 64:128] = Ws[t]
        # rows p = 2h+cs, cols m = cs*64+g
        for cs in range(2):
            hblk[t, cs::2, cs * 64 : cs * 64 + 64] = Hs[t]
    return cstk.astype(bf), wblk.astype(bf), hblk.astype(bf)


def kernel(x, Ws, Hs, Cs, window_size):
    global LAST_EXEC_NS
    import ml_dtypes
    from concourse.bass_utils import run_bass_kernel_spmd

    bf = ml_dtypes.bfloat16
    x = np.asarray(x, dtype=np.float32)
    Ws = np.asarray(Ws, dtype=np.float32)
    Hs = np.asarray(Hs, dtype=np.float32)
    Cs = np.asarray(Cs, dtype=np.float32)
    assert int(window_size) == WS
    assert x.shape == (B, C, H, W)

    nc = _get_nc()
    cstk, wblk, hblk = _prep_consts(Ws, Hs, Cs)
    xb = x.astype(bf)

    in_maps = []
    for core in range(NCORES):
        b, ph = core // 2, core % 2
        shard = np.ascontiguousarray(xb[b, :, ph * PH : (ph + 1) * PH, :])
        in_maps.append({"x": shard, "cstk": cstk, "wblk": wblk, "hblk": hblk})

    res = run_bass_kernel_spmd(nc, in_maps, core_ids=list(range(NCORES)))
    LAST_EXEC_NS = res.exec_time_ns

    full = np.empty((B, C, H, W), dtype=np.float32)
    for core in range(NCORES):
        b, ph = core // 2, core % 2
        full[b, :, ph * PH : (ph + 1) * PH, :] = res.results[core]["out"].astype(
            np.float32
        )
    return full


# revision 28
# speedup vs baseline: 1.3071x; 1.3071x over previous
"""FeatureProcessingBlock Trainium kernel (bf16 pipeline, v5 — PE row-tiled
c-stage + bank-batched PSUM drains).

out = sum_t einsum('bcphqw,twW,thH,tcC->bCpHqW', x.reshape(B,C,P,64,Q,64), Ws, Hs, Cs)

Sharding: 8 cores = (B=4) x (H-halves=2); each core gets x[b, :, ph*256:(ph+1)*256, :]
a [C=48, 256, 512] slab = 4 p-blocks x 4 double-windows (dw = two adjacent
64x64 windows in a 128-wide w-slab).

Per-core pipeline (PSUM evacuation on ACT+DVE is the bottleneck; matmuls are
row-tiled so the PE stays off the critical path; drains are batched to large
free dims and split across both engines by measured rates):

  c-stage  (data-stationary, 2x PE row tiling): two concurrent quadrant
            matmuls T0/T8 contract c=48 for the even/odd h-parity:
            lhsT = x[(par) c48, wp128] chunk at partitions 64*par,
            rhs  = cstk [48, (t3 j24 cs2)=144] (plain Cs stack, no zeros)
            -> one 2-bank PSUM tile holds 3 hh x both parities, hh-interleaved
               so the drain is a single [128, 2par, 432] op
            -> drain/cast to Ybuf [128 (win,w), (t3, j24, cs2, h64)]  (cs-major)
  wT-stage (fused w-matmul + transpose; data-stationary): lhsT = Ybuf (t,j)-chunk
            [128 (win,w), 128 (cs,h)], rhs = blkdiag(Ws_t, Ws_t)
            -> PSUM [128 (cs,h), (win, W')] -- transposed for the h-stage
            -> drain/cast to ZT[t] bf16 (rows cs*64+h)
  h-stage  (t-sum in PSUM): lhsT = Hblk[t] (rows cs*64+h, cols (cs,H')),
            rhs = ZT[t] chunks, 3-matmul accumulation
            -> O [128 (cs, H'), (j8, W')] -> drain f32->bf16 -> DMA out
"""

import sys
import types

import numpy as np


def _ensure_ntff_hook_module():
    """concourse.bass_utils imports antenv.axon_hooks when BASS_TRACE is set;
    provide a fallback module (wired to the ctypes NTFF hook when available)
    so tracing degrades gracefully instead of crashing."""
    try:
        import antenv.axon_hooks  # noqa: F401

        return
    except ImportError:
        pass
    mod = types.ModuleType("antenv.axon_hooks")
    mod._hook = None
    mod.set_axon_ntff_profile_hook = lambda h: setattr(mod, "_hook", h)
    mod.get_axon_ntff_profile_hook = lambda: mod._hook
    sys.modules["antenv.axon_hooks"] = mod
    try:
        from trn_agent_boot.trn_boot import _ntff_profile_via_ctypes

        mod._hook = _ntff_profile_via_ctypes("/opt/axon/libaxon_pjrt.so")
    except Exception:
        pass


_ensure_ntff_hook_module()

B, C, H, W = 4, 48, 512, 512
T, WS = 3, 64
NCORES = 8
PH = H // 2    # 256 rows per core
NP = PH // 64  # 4 p-blocks

LAST_EXEC_NS = None
_CACHE = {}


def _build():
    import concourse.bacc as bacc
    import concourse.mybir as mybir
    from concourse.bass import MemorySpace
    from concourse.tile import TileContext

    F32 = mybir.dt.float32
    BF16 = mybir.dt.bfloat16

    nc = bacc.Bacc("TRN2", target_bir_lowering=False, debug=False, num_devices=NCORES)
    x = nc.dram_tensor("x", [C, PH, W], BF16, kind="ExternalInput")
    cstk = nc.dram_tensor("cstk", [128, T * 24 * 2], BF16, kind="ExternalInput")
    wblk = nc.dram_tensor("wblk", [T, 128, 128], BF16, kind="ExternalInput")
    hblk = nc.dram_tensor("hblk", [T, 128, 128], BF16, kind="ExternalInput")
    out = nc.dram_tensor("out", [C, PH, W], BF16, kind="ExternalOutput")

    with TileContext(nc) as tc:
        with (
            tc.tile_pool(name="consts", bufs=1) as consts,
            tc.tile_pool(name="xin", bufs=2) as xin,
            tc.tile_pool(name="xfirst", bufs=1) as xfirst,
            tc.tile_pool(name="ybuf", bufs=3) as ypool,
            tc.tile_pool(name="ztbuf", bufs=2) as ztpool,
            tc.tile_pool(name="obuf", bufs=2) as opool,
            tc.tile_pool(name="cps", bufs=2, space=MemorySpace.PSUM) as cps,
            tc.tile_pool(name="tps", bufs=3, space=MemorySpace.PSUM) as tps,
            tc.tile_pool(name="ops", bufs=1, space=MemorySpace.PSUM) as ops,
        ):
            # Cs stack, rows 0-47 (for quadrant T0) and 64-111 (T8)
            cstk_sb = consts.tile([128, T, 24, 2], BF16)
            nc.sync.dma_start(
                out=cstk_sb,
                in_=cstk[:, :].rearrange("k (t j s) -> k t j s", t=T, j=24),
            )
            wblk_sb = consts.tile([128, T, 128], BF16)
            hblk_sb = consts.tile([128, T, 128], BF16)
            wh_loaded = []

            def load_wh():
                nc.sync.dma_start(
                    out=wblk_sb, in_=wblk[:, :, :].rearrange("t k m -> k t m")
                )
                nc.sync.dma_start(
                    out=hblk_sb, in_=hblk[:, :, :].rearrange("t k m -> k t m")
                )
                wh_loaded.append(True)

            obs = {}

            # drain engine round-robin: drains are the global bottleneck, so
            # keep both engines fed; ACT is faster (1.2 vs 0.96 GHz) so it
            # gets a bigger share.
            def emit_c_stage(xt, yb, d):
                """c-stage for one block: 32 hh rows x 2 parities, grouped 3 hh
                per 2-bank PSUM tile (hh-interleaved cols) -> 11 batched drains."""
                wp0 = 128 * (d % 2)
                chunks = []
                groups = [(3 * g, min(3, 32 - 3 * g)) for g in range((32 + 2) // 3)]
                for gi, (hh0, nh) in enumerate(groups):
                    def c_group(hh0=hh0, nh=nh, gi=gi, yb=yb, wp0=wp0):
                        cpt = cps.tile([128, 1024], F32, tag="c")
                        pv = cpt[:, :].rearrange("p (par x) -> p par x", par=2)
                        for i in range(nh):
                            xq, r = xt[hh0 + i]
                            for par in range(2):
                                v = pv[:, par, 144 * i : 144 * i + 144].rearrange(
                                    "p (t j s) -> p t j s", t=T, j=24
                                )
                                nc.tensor.matmul(
                                    v,
                                    lhsT=xq[64 * par : 64 * par + 48, r, wp0 : wp0 + 128],
                                    rhs=cstk_sb[64 * par : 64 * par + 48],
                                    start=True,
                                    stop=True,
                                )
                        # batched per-parity drains (3D-fusable APs): src
                        # (tj, hh, cs) -> yb[..., h, cs] with h = 2*(hh0+hh)+par
                        for par in range(2):
                            src = pv[:, par, 0 : 144 * nh].rearrange(
                                "p (h t j s) -> p (t j) h s", t=T, j=24, s=2
                            )
                            dst = yb[
                                :, :, :, 2 * hh0 + par : 2 * hh0 + 2 * nh : 2, :
                            ].rearrange("p t j h s -> p (t j) h s")
                            nc.any.tensor_copy(out=dst, in_=src)
                    chunks.append(c_group)
                return chunks

            def make_groups(p, d, yb, last=False):
                """wT + h + DMA emission callbacks for one block."""
                groups = []
                ztbs = [
                    ztpool.tile([128, 24, 128], BF16, tag=f"zt{t}", name=f"ztb{t}")
                    for t in range(T)
                ]
                ob = obs[p]

                def wt_group(t, jq, yb=yb):
                    ztb = ztbs[t]
                    tp = tps.tile([128, 4, 128], F32, tag="t")
                    for i in range(4):
                        nc.tensor.matmul(
                            tp[:, i],
                            lhsT=yb[:, t, 4 * jq + i],
                            rhs=wblk_sb[:, t, :],
                            start=True,
                            stop=True,
                        )
                    nc.any.tensor_copy(out=ztb[:, 4 * jq : 4 * jq + 4, :], in_=tp)

                def h_group(cc, win, d=d):
                    op = ops.tile([128, 8, 64], F32, tag="o")
                    for t in range(T):
                        nc.tensor.matmul(
                            op,
                            lhsT=hblk_sb[:, t, :],
                            rhs=ztbs[t][
                                :, 8 * cc : 8 * cc + 8, 64 * win : 64 * win + 64
                            ],
                            start=(t == 0),
                            stop=(t == T - 1),
                        )
                    nc.any.tensor_copy(out=ob[:, cc, :, d, win, :], in_=op)

                def out_dma(a, p=p, d=d):
                    if d != 3:
                        return
                    for cs in range(2):
                        nc.sync.dma_start(
                            out=out[
                                16 * a + cs : 16 * a + 16 : 2,
                                64 * p : 64 * p + 64,
                                :,
                            ].rearrange("c h w -> h c w"),
                            in_=ob[
                                64 * cs : 64 * cs + 64, a, :, :
                            ].rearrange("p j d win w -> p j (d win w)"),
                        )

                # jq-major so h-stage cc becomes ready after jq=2cc+1; weave
                # h/dma groups right after their inputs so O-drains spread out
                import functools
                for jq in range(6):
                    for t in range(T):
                        groups.append(functools.partial(wt_group, t, jq))
                    if jq % 2 == 1:
                        cc = jq // 2
                        groups.append(functools.partial(h_group, cc, 0))
                        groups.append(functools.partial(h_group, cc, 1))
                        groups.append(functools.partial(out_dma, cc))
                return groups

            prev_groups = []
            xts = {}
            blocks = [(p, d) for p in range(NP) for d in range(4)]

            def load_x(k):
                p, d = blocks[k]
                if d % 2 != 0:
                    return
                # ---- load half p-row in hh-pieces: [par@{0,64} c48, nh, 256 w]
                sizes = [3, 3, 2, 8, 8, 8] if k == 0 else [8, 8, 8, 8]
                hhmap = []
                hh0 = 0
                for q, nh in enumerate(sizes):
                    pool = xfirst if k == 0 else xin
                    xq = pool.tile(
                        [128, nh, 256], BF16, tag=f"x{q}", name=f"x{k}_{q}"
                    )
                    h0 = 64 * p + 2 * hh0
                    for par in range(2):
                        nc.sync.dma_start(
                            out=xq[64 * par : 64 * par + 48],
                            in_=x[
                                :,
                                h0 + par : h0 + 2 * nh : 2,
                                256 * (d // 2) : 256 * (d // 2) + 256,
                            ],
                        )
                    for r in range(nh):
                        hhmap.append((xq, r))
                    hh0 += nh
                xts[p, d // 2] = hhmap

            load_x(0)
            load_x(1)
            for k, (p, d) in enumerate(blocks):
                if k + 2 < len(blocks):
                    load_x(k + 2)
                if d % 2 == 0 and not wh_loaded:
                    load_wh()
                if d == 0:
                    obs[p] = opool.tile(
                        [128, 3, 8, 4, 2, 64], BF16, tag="ob", name=f"ob{p}"
                    )
                # Ybuf [128 (win,w), (t3, j24, h64, cs2)]
                yb = ypool.tile([128, T, 24, 64, 2], BF16, tag="y")
                chunks = emit_c_stage(xts[p, d // 2], yb, d)
                # proportionally interleave this block's c-stage with the
                # previous block's wT+h groups (no tail bursts)
                nc_, ng = len(chunks), len(prev_groups)
                for i in range(nc_):
                    chunks[i]()
                    for g in prev_groups[(i * ng) // nc_ : ((i + 1) * ng) // nc_]:
                        g()
                prev_groups = make_groups(p, d, yb)
            for g in prev_groups:
                g()

    nc.compile()
    return nc


def _get_nc():
    if "nc" not in _CACHE:
        _CACHE["nc"] = _build()
    return _CACHE["nc"]


def _prep_consts(Ws, Hs, Cs):
    import ml_dtypes

    bf = ml_dtypes.bfloat16
    # cstk [128, (t3, j24, cs2)]: rows 0-47 and 64-111 hold the plain Cs stack
    # (c, (t, j, cs)) with C' = 2j+cs
    cstk = np.zeros((128, T, 24, 2), np.float32)
    cst = Cs.transpose(1, 0, 2).reshape(C, T, 24, 2)
    cstk[0:48] = cst
    cstk[64:112] = cst
    cstk = cstk.reshape(128, T * 24 * 2)
    wblk = np.zeros((T, 128, 128), np.float32)
    hblk = np.zeros((T, 128, 128), np.float32)
    for t in range(T):
        wblk[t, 0:64, 0:64] = Ws[t]
        wblk[t, 64:128, 64:128] = Ws[t]
        # rows p = 2h+cs, cols m = cs*64+g
        for cs in range(2):
            hblk[t, cs::2, cs * 64 : cs * 64 + 64] = Hs[t]
    return cstk.astype(bf), wblk.astype(bf), hblk.astype(bf)


def kernel(x, Ws, Hs, Cs, window_size):
    global LAST_EXEC_NS
    import ml_dtypes
    from concourse.bass_utils import run_bass_kernel_spmd

    bf = ml_dtypes.bfloat16
    x = np.asarray(x, dtype=np.float32)
    Ws = np.asarray(Ws, dtype=np.float32)
    Hs = np.asarray(Hs, dtype=np.float32)
    Cs = np.asarray(Cs, dtype=np.float32)
    assert int(window_size) == WS
    assert x.shape == (B, C, H, W)

    nc = _get_nc()
    cstk, wblk, hblk = _prep_consts(Ws, Hs, Cs)
    xb = x.astype(bf)

    in_maps = []
    for core in range(NCORES):
        b, ph = core // 2, core % 2
        shard = np.ascontiguousarray(xb[b, :, ph * PH : (ph + 1) * PH, :])
        in_maps.append({"x": shard, "cstk": cstk, "wblk": wblk, "hblk": hblk})

    res = run_bass_kernel_spmd(nc, in_maps, core_ids=list(range(NCORES)))
    LAST_EXEC_NS = res.exec_time_ns

    full = np.empty((B, C, H, W), dtype=np.float32)
    for core in range(NCORES):
        b, ph = core // 2, core % 2
        full[b, :, ph * PH : (ph + 1) * PH, :] = res.results[core]["out"].astype(
            np.float32
        )
    return full
